# revision 13
# baseline (speedup 1.0000x reference)
"""Trainium2 Bass kernel for the CLIT-style sparse local attention module.

Strategy (8 NeuronCores, SPMD):
  - core c handles batch b = c // 4 and query chunk qc = c % 4 (1024 queries).
  - The 5 convs are SPLIT 4-way within each batch group: core computes a
    20-row strip (its 16 pixel rows + halo) selected purely by per-core
    input data (im2col slice + row mask), writes its kv/q row slices, then
    a 4-core AllGather assembles the full pixel-major maps in DRAM:
    kv_rows [4486, 384] (k|v per pixel, e-major channels, 3-row apron) and
    q_rows [4226, 256] (q channels e-major + fp32-as-2xbf16 image).
  - k/q/v channels are stored E-MAJOR (c' = e*8 + h) so the attention
    elementwise ops run in the DVE 2x packed mode with the head axis
    innermost.
  - The 7x7 window gather runs as dma_gather of 7-pixel row segments.
  - Attention per 128-query tile: q via scalar_tensor_tensor chain,
    logits = one big k*q mul + a 5-level tree reduce over e, softmax on
    DVE/ACT, weighted-v muls (DVE + gpsimd), PE-transposed per-dy into
    xt blocks that accumulate straight into the MLP0 PSUM (no full-xt
    staging barrier).
  - Host precomputes gather indices / bilinear weights / masks and all
    weight layout shuffles.
"""

import sys

sys.path.insert(0, "/opt/trn_rl_repo")

import numpy as np
import ml_dtypes

# ---------------- problem constants (hardcoded per contract) ----------------
B, CH_IN, H, W = 2, 3, 64, 64
Q = 4096
DIM, HEAD, R = 192, 8, 3
RR = 2 * R + 1
RA = RR * RR          # 49
HD = DIM // HEAD      # 24
ENC = 64
HID = 256
P = H * W             # 4096 pixels
N_CORES = 8
QC = Q * B // N_CORES  # 1024 queries per core
NT = QC // 128         # 8 query tiles per core

KV_ELEM = 2 * DIM                 # 384 (k row + v row, bf16)
KV_PAD = 195                      # 3 rows + 3 px apron before pixel 0
KV_ROWS = P + 2 * KV_PAD          # 4486
Q_PAD = 65                        # 1 row + 1 px apron
Q_ROWS = P + 2 * Q_PAD            # 4226
DYW = RR * DIM                    # 1344 columns per window-row chunk
DY_BLOCKS = 11                    # ceil(1344 / 128)
KBLK = RR * DY_BLOCKS             # 77 K-blocks for MLP layer 0

# conv strip: 16 output rows + halo -> 20 enc rows starting at 16j-2
SROWS = 16            # pixel rows per core
EROWS = 20            # enc rows computed (16 + 2 halo each side)
COLN = EROWS * W      # 1280 im2col columns

f32 = np.float32
bf16 = ml_dtypes.bfloat16

_PROGRAM = None  # cached compiled Bass program


# ============================ device program ================================

def build_program():
    import concourse.bass as bass
    import concourse.tile as tile
    from concourse import bacc, mybir

    dt = mybir.dt

    nc = bacc.Bacc("TRN2", target_bir_lowering=False, debug=False,
                   enable_asserts=False, num_devices=N_CORES)

    def din(name, shape, dtype):
        return nc.dram_tensor(name, list(shape), dtype, kind="ExternalInput").ap()

    # ---- inputs (per-core data) ----
    inp_col = din("inp_col", [28, COLN], dt.bfloat16)
    inp_hilo = din("inp_hilo", [128, 8, 6], dt.bfloat16)
    rowmask = din("rowmask", [128, EROWS], dt.bfloat16)
    w_enc = din("w_enc", [28, ENC], dt.bfloat16)
    w_chp = din("w_chp", [128, 3, DIM], dt.bfloat16)
    w_ch2 = din("w_ch2", [ENC, 3, DIM], dt.bfloat16)
    w_qkv0 = din("w_qkv0", [128, 3, 9, DIM], dt.bfloat16)
    w_qkv1p = din("w_qkv1p", [128, 3, 3, DIM], dt.bfloat16)
    w_qkv1k2 = din("w_qkv1k2", [64, 3, 3, DIM], dt.bfloat16)
    qkvb = din("qkvb", [128, 6], dt.float32)
    ch_b = din("ch_b", [128, 2], dt.float32)
    m0w = din("m0w", [128, KBLK, HID], dt.bfloat16)
    m13w = din("m13w", [128, 6, HID], dt.bfloat16)
    m4w = din("m4w", [128, 2, 3], dt.bfloat16)
    bmlp = din("bmlp", [128, 8], dt.float32)
    b4 = din("b4", [128, 3], dt.float32)
    ident = din("ident", [128, 128], dt.bfloat16)
    kvidx = din("kvidx", [128, NT, RR * 8], dt.int16)
    qidx = din("qidx", [128, 128], dt.int16)
    maskt = din("maskt", [128, NT, RA], dt.float32)
    nmaskt = din("nmaskt", [128, NT], dt.float32)
    qwt = din("qwt", [128, NT, 4], dt.float32)
    qwbt = din("qwbt", [128, NT, 4], dt.float32)
    out = nc.dram_tensor("out", [QC, 3], dt.float32, kind="ExternalOutput").ap()

    with tile.TileContext(nc) as tc:
        with tc.tile_pool(name="dram", bufs=1, space="DRAM") as dp:
            kv_rows = dp.tile([KV_ROWS, KV_ELEM], dt.bfloat16)
            q_rows = dp.tile([Q_ROWS, 256], dt.bfloat16)
            kv_mine = dp.tile([SROWS * W, KV_ELEM], dt.bfloat16)
            q_mine = dp.tile([SROWS * W, 256], dt.bfloat16)

            _convs(nc, tc, mybir, locals())
            _attention(nc, tc, mybir, locals())

    nc.compile()
    return nc


def _convs(nc, tc, mybir, env):
    dt = mybir.dt
    AF = mybir.ActivationFunctionType

    inp_col, w_enc = env["inp_col"], env["w_enc"]
    w_chp, w_ch2 = env["w_chp"], env["w_ch2"]
    w_qkv0, w_qkv1p, w_qkv1k2 = env["w_qkv0"], env["w_qkv1p"], env["w_qkv1k2"]
    qkvb = env["qkvb"]
    ch_b, inp_hilo, rowmask = env["ch_b"], env["inp_hilo"], env["rowmask"]
    kv_rows, q_rows = env["kv_rows"], env["q_rows"]
    kv_mine, q_mine = env["kv_mine"], env["q_mine"]

    with (
        tc.tile_pool(name="cw", bufs=1) as cw,
        tc.tile_pool(name="cfeat", bufs=1) as cf,
        tc.tile_pool(name="cpsum", bufs=2, space="PSUM") as cp,
        tc.tile_pool(name="qpsum", bufs=2, space="PSUM") as cpq,
        tc.tile_pool(name="ctpsum", bufs=2, space="PSUM") as cpt,
        tc.tile_pool(name="cstage", bufs=3) as cs,
    ):
        col_sb = cw.tile([28, COLN], dt.bfloat16)
        nc.sync.dma_start(col_sb[:], inp_col)
        wenc_sb = cw.tile([28, ENC], dt.bfloat16)
        nc.sync.dma_start(wenc_sb[:], w_enc)
        wchp_sb = cw.tile([128, 3, DIM], dt.bfloat16)
        nc.sync.dma_start(wchp_sb[:], w_chp)
        wch2_sb = cw.tile([ENC, 3, DIM], dt.bfloat16)
        nc.sync.dma_start(wch2_sb[:], w_ch2)
        wq0 = cw.tile([128, 3, 9, DIM], dt.bfloat16)
        nc.sync.dma_start(wq0[:], w_qkv0)
        wq1p = cw.tile([128, 3, 3, DIM], dt.bfloat16)
        nc.sync.dma_start(wq1p[:], w_qkv1p)
        wq1k2 = cw.tile([64, 3, 3, DIM], dt.bfloat16)
        nc.sync.dma_start(wq1k2[:], w_qkv1k2)
        qkvb_sb = cw.tile([128, 6], dt.float32)
        nc.sync.dma_start(qkvb_sb[:], qkvb)
        chb_sb = cw.tile([128, 2], dt.float32)
        nc.sync.dma_start(chb_sb[:], ch_b)
        hilo_sb = cw.tile([128, 8, 6], dt.bfloat16)
        nc.sync.dma_start(hilo_sb[:], inp_hilo)
        rm_sb = cw.tile([128, EROWS], dt.bfloat16)
        nc.sync.dma_start(rm_sb[:], rowmask)
        id_c = cw.tile([128, 128], dt.bfloat16)
        nc.sync.dma_start(id_c[:], env["ident"])

        # zero the DRAM row aprons (kv: 195 rows x 384 = 128x585; q: 65x256)
        zt = cw.tile([128, 585], dt.bfloat16)
        nc.vector.memset(zt[:], 0.0)
        kvf = kv_rows[:, :].flatten()
        nc.sync.dma_start(kvf[0: KV_PAD * KV_ELEM]
                          .rearrange("(p a) -> p a", p=128), zt[:])
        nc.sync.dma_start(kvf[(KV_PAD + P) * KV_ELEM: KV_ROWS * KV_ELEM]
                          .rearrange("(p a) -> p a", p=128), zt[:, 0:585])
        qf = q_rows[:, :].flatten()
        nc.sync.dma_start(qf[0: Q_PAD * 256]
                          .rearrange("(p a) -> p a", p=128), zt[:, 0:130])
        nc.sync.dma_start(qf[(Q_PAD + P) * 256: Q_ROWS * 256]
                          .rearrange("(p a) -> p a", p=128), zt[:, 0:130])

        encp = cf.tile([ENC, EROWS, 66], dt.bfloat16)
        nc.vector.memset(encp[:], 0.0)
        fp0 = cf.tile([128, EROWS, 66], dt.bfloat16)
        nc.vector.memset(fp0[:], 0.0)
        fp1 = cf.tile([64, EROWS, 66], dt.bfloat16)
        nc.vector.memset(fp1[:], 0.0)
        encb2 = cf.tile([128, EROWS, 66], dt.bfloat16)
        fp1b = cf.tile([128, EROWS, 66], dt.bfloat16)

        # ---- enc conv (bias + border-zero folded into im2col row 27) ----
        for (r0, nr) in ((0, 8), (8, 8), (16, 4)):
            ps = cp.tile([128, 512], dt.float32, tag="cps")
            nc.tensor.matmul(ps[:ENC, 0:nr * 64], wenc_sb[:],
                             col_sb[:, r0 * 64:(r0 + nr) * 64],
                             start=True, stop=True)
            nc.scalar.copy(encp[:, r0:r0 + nr, 1:65],
                           ps[:ENC, 0:nr * 64].rearrange("p (a b) -> p a b", a=nr))

        # encb2: enc features with a one-column-shifted copy in partitions 64:
        nc.vector.tensor_copy(encb2[0:64, :, :], encp[:])
        nc.vector.tensor_copy(encb2[64:128, :, 0:65], encp[:, :, 1:66])

        # ---- ch conv rows 1..18 (kx 0/1 paired into K=128, kx=2 single) ----
        for (r0, nr) in ((1, 8), (9, 8), (17, 2)):
            for m, msz in ((0, 128), (1, 64)):
                ps = cp.tile([128, 512], dt.float32, tag="cps")
                for ky in range(3):
                    rhs = encb2[:, r0 + ky - 1: r0 + ky - 1 + nr, 0:64]
                    nc.tensor.matmul(ps[:msz, 0:nr * 64],
                                     wchp_sb[:, ky, m * 128: m * 128 + msz],
                                     rhs, start=(ky == 0), stop=False)
                for ky in range(3):
                    rhs = encp[:, r0 + ky - 1: r0 + ky - 1 + nr, 2:66]
                    nc.tensor.matmul(ps[:msz, 0:nr * 64],
                                     wch2_sb[:, ky, m * 128: m * 128 + msz],
                                     rhs, start=False, stop=(ky == 2))
                dstp = (fp0 if m == 0 else fp1)
                dst = dstp[:msz, r0:r0 + nr, 1:65]
                nc.scalar.activation(dst,
                                     ps[:msz, 0:nr * 64]
                                     .rearrange("p (a b) -> p a b", a=nr),
                                     AF.Identity, bias=chb_sb[:msz, m: m + 1])

        # zero out-of-image rows so the qkv conv sees 'SAME' padding
        nc.vector.tensor_mul(fp0[:], fp0[:],
                             rm_sb[:].unsqueeze(2).broadcast_to((128, EROWS, 66)))
        nc.vector.tensor_mul(fp1[:], fp1[:],
                             rm_sb[0:64].unsqueeze(2).broadcast_to((64, EROWS, 66)))

        # fp1b: channel-chunk-1 features with one-column-shifted copy
        nc.vector.tensor_copy(fp1b[0:64, :, :], fp1[:])
        nc.vector.tensor_copy(fp1b[64:128, :, 0:65], fp1[:, :, 1:66])

        # ---- q/k/v convs + PE transpose to pixel-major rows.  The q conv
        # runs first so its AllGather overlaps the k/v convs; the kv
        # AllGather then overlaps the attention phase's q-side prework. ----
        groups = [[0, 1, 2, 3], [4, 5, 6, 7]]

        def conv_ci(ci, g, dstg, coff0):
            vr0 = 2 + 8 * g                   # strip row of first output row
            for m, msz in ((0, 128), (1, 64)):
                ps = cpq.tile([128, 512], dt.float32, tag="qkvps")
                for off in range(9):
                    ky, kx = off // 3, off % 3
                    rhs0 = fp0[:, vr0 + ky - 1: vr0 + ky - 1 + 8, kx: kx + 64]
                    nc.tensor.matmul(ps[:msz, :],
                                     wq0[:, ci, off, m * 128: m * 128 + msz],
                                     rhs0, start=(off == 0), stop=False)
                for ky in range(3):
                    rhs1 = fp1b[:, vr0 + ky - 1: vr0 + ky - 1 + 8, 0:64]
                    nc.tensor.matmul(ps[:msz, :],
                                     wq1p[:, ci, ky, m * 128: m * 128 + msz],
                                     rhs1, start=False, stop=False)
                for ky in range(3):
                    rhs1 = fp1[:, vr0 + ky - 1: vr0 + ky - 1 + 8, 2:66]
                    nc.tensor.matmul(ps[:msz, :],
                                     wq1k2[:, ci, ky, m * 128: m * 128 + msz],
                                     rhs1, start=False, stop=(ky == 2))
                csb = cs.tile([128, 512], dt.bfloat16, tag="convsb")
                nc.scalar.activation(csb[:msz, :], ps[:msz, :], AF.Identity,
                                     bias=qkvb_sb[:msz, ci * 2 + m: ci * 2 + m + 1])
                tps = cpt.tile([128, 512], dt.bfloat16, tag="ctps")
                for blk in range(4):
                    nc.tensor.transpose(
                        tps[:, blk * 128: blk * 128 + msz],
                        csb[:msz, blk * 128:(blk + 1) * 128],
                        id_c[:msz, :msz])
                nc.scalar.copy(
                    dstg[:, :, coff0 + m * 128: coff0 + m * 128 + msz],
                    tps[:].rearrange("p (a b) -> p a b", a=4)[:, :, 0:msz])

        for g in range(2):
            qstage = cs.tile([128, 4, 256], dt.bfloat16, tag="qstage")
            nc.vector.memset(qstage[:, :, 198:], 0.0)
            nc.vector.tensor_copy(qstage[:, :, 192:198],
                                  hilo_sb[:, g * 4:(g + 1) * 4, :])
            conv_ci(0, g, qstage, 0)
            nc.sync.dma_start(
                q_mine[g * 512:(g + 1) * 512, :]
                .rearrange("(b p) e -> p b e", p=128), qstage[:])
        nc.gpsimd.collective_compute(
            "AllGather", mybir.AluOpType.bypass,
            replica_groups=groups,
            ins=[q_mine[:, :].opt()],
            outs=[q_rows[Q_PAD: Q_PAD + P, :].opt()],
        )

        for g in range(2):
            kvstage = cs.tile([128, 4, KV_ELEM], dt.bfloat16, tag="kvstage")
            conv_ci(1, g, kvstage, 0)
            conv_ci(2, g, kvstage, DIM)
            nc.sync.dma_start(
                kv_mine[g * 512:(g + 1) * 512, :]
                .rearrange("(b p) e -> p b e", p=128), kvstage[:])
        nc.gpsimd.collective_compute(
            "AllGather", mybir.AluOpType.bypass,
            replica_groups=groups,
            ins=[kv_mine[:, :].opt()],
            outs=[kv_rows[KV_PAD: KV_PAD + P, :].opt()],
        )


def _attention(nc, tc, mybir, env):
    import concourse.bass as bass
    dt = mybir.dt
    AX = mybir.AxisListType
    AF = mybir.ActivationFunctionType
    ALU = mybir.AluOpType

    kv_rows, q_rows = env["kv_rows"], env["q_rows"]
    m0w, m13w, m4w = env["m0w"], env["m13w"], env["m4w"]
    bmlp, b4, ident = env["bmlp"], env["b4"], env["ident"]
    kvidx, qidx = env["kvidx"], env["qidx"]
    maskt, qwt, qwbt = env["maskt"], env["qwt"], env["qwbt"]
    nmaskt = env["nmaskt"]
    out = env["out"]

    with (
        tc.tile_pool(name="aw", bufs=1) as aw,
        tc.tile_pool(name="gath", bufs=3) as gp,
        tc.tile_pool(name="attn", bufs=2) as ap_,
        tc.tile_pool(name="abp", bufs=2) as abp,
        tc.tile_pool(name="wvp", bufs=4) as wvp,
        tc.tile_pool(name="xtp", bufs=2) as xtp,
        tc.tile_pool(name="hp", bufs=2) as hp,
        tc.tile_pool(name="outp", bufs=1) as op_,
        tc.tile_pool(name="tpsum", bufs=3, space="PSUM") as tp_,
        tc.tile_pool(name="mpsum", bufs=2, space="PSUM") as mp_,
        tc.tile_pool(name="m2psum", bufs=2, space="PSUM") as mp2_,
        tc.tile_pool(name="opsum", bufs=1, space="PSUM") as osp,
    ):
        m0w_sb = aw.tile([128, KBLK, HID], dt.bfloat16)
        nc.sync.dma_start(m0w_sb[:], m0w)
        m13_sb = aw.tile([128, 6, HID], dt.bfloat16)
        nc.sync.dma_start(m13_sb[:], m13w)
        m4_sb = aw.tile([128, 2, 3], dt.bfloat16)
        nc.sync.dma_start(m4_sb[:], m4w)
        bm_sb = aw.tile([128, 8], dt.float32)
        nc.sync.dma_start(bm_sb[:], bmlp)
        b4_sb = aw.tile([128, 3], dt.float32)
        nc.sync.dma_start(b4_sb[:], b4)
        id_sb = aw.tile([128, 128], dt.bfloat16)
        nc.sync.dma_start(id_sb[:], ident)
        kvi_sb = aw.tile([128, NT, RR * 8], dt.int16)
        nc.sync.dma_start(kvi_sb[:], kvidx)
        qi_sb = aw.tile([128, 128], dt.int16)
        nc.sync.dma_start(qi_sb[:], qidx)
        mk_sb = aw.tile([128, NT, RA], dt.float32)
        nc.sync.dma_start(mk_sb[:], maskt)
        nm_sb = aw.tile([128, NT], dt.float32)
        nc.sync.dma_start(nm_sb[:], nmaskt)
        qw_sb = aw.tile([128, NT, 4], dt.float32)
        nc.sync.dma_start(qw_sb[:], qwt)
        qwb_sb = aw.tile([128, NT, 4], dt.float32)
        nc.sync.dma_start(qwb_sb[:], qwbt)

        out_sb = op_.tile([128, NT, 3], dt.float32)

        qv_ap = q_rows[:, :]
        qv_ap = bass.AP(qv_ap.tensor, qv_ap.offset,
                        [[256, Q_ROWS - 1], [1, 512]])
        kv_ap = kv_rows[:, :]
        kv_ap = bass.AP(kv_ap.tensor, kv_ap.offset,
                        [[KV_ELEM, KV_ROWS - 6], [1, RR * KV_ELEM]])

        # ---- q-side prework for ALL tiles: one big bilinear-corner gather
        # plus the weighted-sum chains.  Runs while the kv AllGather is in
        # flight (it only depends on the q AllGather). ----
        qgb = aw.tile([128, NT * 2, 512], dt.bfloat16)
        nc.gpsimd.dma_gather(qgb[:], qv_ap, qi_sb[:],
                             num_idxs=NT * 256, num_idxs_reg=NT * 256,
                             elem_size=512, elem_step=256, single_packet=False)
        qgb4 = qgb[:].rearrange("p (t y) c -> p t y c", y=2)
        qb_all = aw.tile([128, NT, DIM], dt.bfloat16)
        base_all = aw.tile([128, NT, 3], dt.float32)
        for t in range(NT):
            qacc = ap_.tile([128, DIM], dt.float32, tag="qacc")
            nc.vector.tensor_scalar_mul(qacc[:], qgb4[:, t, 0, 0:DIM],
                                        qw_sb[:, t, 0:1])
            for i, (y, x) in enumerate(((0, 1), (1, 0), (1, 1))):
                nc.vector.scalar_tensor_tensor(
                    qacc[:], qgb4[:, t, y, x * 256: x * 256 + DIM],
                    qw_sb[:, t, i + 1:i + 2], qacc[:], ALU.mult, ALU.add)
            nc.scalar.copy(qb_all[:, t, :], qacc[:])
            bacc = ap_.tile([128, 6], dt.float32, tag="bacc")
            nc.vector.tensor_scalar_mul(bacc[:], qgb4[:, t, 0, 192:198],
                                        qwb_sb[:, t, 0:1])
            for i, (y, x) in enumerate(((0, 1), (1, 0), (1, 1))):
                nc.vector.scalar_tensor_tensor(
                    bacc[:], qgb4[:, t, y, x * 256 + 192: x * 256 + 198],
                    qwb_sb[:, t, i + 1:i + 2], bacc[:], ALU.mult, ALU.add)
            nc.vector.tensor_add(base_all[:, t, :], bacc[:, 0:3], bacc[:, 3:6])

        def issue_gathers(t):
            kvg = gp.tile([128, RR, RR * KV_ELEM], dt.bfloat16, tag="kvg")
            nc.gpsimd.dma_gather(kvg[:], kv_ap, kvi_sb[:, t, :],
                                 num_idxs=RR * 128, num_idxs_reg=RR * 128,
                                 elem_size=RR * KV_ELEM, elem_step=KV_ELEM,
                                 single_packet=False)
            return kvg

        pend = [issue_gathers(0), issue_gathers(1)]
        ctx = [None, None]
        for t in range(NT):
            kvg = pend.pop(0)
            kv5 = kvg[:].rearrange("p y (x c) -> p y x c", x=RR)
            qb = qb_all[:, t, :]

            # -------- logits: k*q mul (in place in the gather buffer) -------
            ek = kv5[:, :, :, 0:DIM]
            nc.vector.tensor_mul(
                ek.rearrange("p y x (e h) -> p y x e h", h=HEAD),
                ek.rearrange("p y x (e h) -> p y x e h", h=HEAD),
                qb.rearrange("p (e h) -> p e h", h=HEAD)
                .unsqueeze(1).unsqueeze(1)
                .broadcast_to((128, RR, RR, HD, HEAD)))
            # tree reduce over e (pairs stay 2x-packed on the h axis)
            e4 = ek.rearrange("p y x (e h) -> p y x e h", h=HEAD)
            nc.vector.tensor_add(e4[:, :, :, 0:12, :], e4[:, :, :, 0:12, :],
                                 e4[:, :, :, 12:24, :])
            nc.vector.tensor_add(e4[:, :, :, 0:6, :], e4[:, :, :, 0:6, :],
                                 e4[:, :, :, 6:12, :])
            nc.vector.tensor_add(e4[:, :, :, 0:3, :], e4[:, :, :, 0:3, :],
                                 e4[:, :, :, 3:6, :])
            e3 = ek.rearrange("p y x (e h) -> p (y x) e h", h=HEAD)
            logits = ap_.tile([128, RA, HEAD], dt.float32, tag="logits")
            nc.vector.tensor_add(logits[:], e3[:, :, 0, :], e3[:, :, 1, :])
            nc.vector.tensor_add(logits[:], logits[:], e3[:, :, 2, :])

            # ------------- softmax (mask folded in additively) --------------
            mask_bc = mk_sb[:, t, :].unsqueeze(2).broadcast_to((128, RA, HEAD))
            nc.vector.tensor_add(logits[:], logits[:], mask_bc)
            elog = ap_.tile([128, RA, HEAD], dt.float32, tag="elog")
            nc.scalar.activation(elog[:], logits[:], AF.Exp)
            ssum = ap_.tile([128, HEAD], dt.float32, tag="ssum")
            nc.vector.reduce_sum(ssum[:], elog[:].rearrange("p r h -> p h r"),
                                 axis=AX.X)
            # reference counts exp(0)=1 for each out-of-image window slot
            nc.vector.tensor_scalar_add(ssum[:], ssum[:], nm_sb[:, t: t + 1])
            rec = ap_.tile([128, HEAD], dt.float32, tag="rec")
            nc.vector.reciprocal(rec[:], ssum[:])
            attnb = abp.tile([128, RA, HEAD], dt.bfloat16, tag="attnb")
            nc.vector.tensor_mul(attnb[:], elog[:],
                                 rec[:].unsqueeze(1).broadcast_to((128, RA, HEAD)))

            ctx[t % 2] = (kvg, kv5, attnb)
            if t + 2 < NT:
                pend.append(issue_gathers(t + 2))
            if t % 2 == 0:
                continue

            # ------- pair: weighted v -> per-dy transpose + MLP0 (N=256) ----
            ps0 = mp_.tile([128, HID], dt.float32, tag="ps0", bufs=1)
            ps1 = mp_.tile([128, HID], dt.float32, tag="ps1", bufs=1)
            xts = {}
            for dy in range(RR + 1):
                if dy < RR:
                    xt = xtp.tile([128, DY_BLOCKS, HID], dt.bfloat16)
                    for u in range(2):
                        _, kv5u, attnbu = ctx[u]
                        wv = wvp.tile([128, DYW], dt.bfloat16, tag="wv")
                        nc.vector.tensor_mul(
                            wv[:].rearrange("p (x e h) -> p x e h",
                                            x=RR, h=HEAD),
                            kv5u[:, dy, :, DIM:2 * DIM]
                            .rearrange("p x (e h) -> p x e h", h=HEAD),
                            attnbu[:, dy * RR:(dy + 1) * RR, :]
                            .unsqueeze(2).broadcast_to((128, RR, HD, HEAD)))
                        for grp, blks in ((0, (0, 1, 2, 3)),
                                          (1, (4, 5, 6, 7)),
                                          (2, (8, 9, 10))):
                            tps = tp_.tile([128, 512], dt.bfloat16, tag="tps")
                            for bi, blk in enumerate(blks):
                                bw = 64 if blk == 10 else 128
                                nc.tensor.transpose(
                                    tps[0:bw, bi * 128: bi * 128 + 128],
                                    wv[:, blk * 128: blk * 128 + bw],
                                    id_sb[:])
                            uc = u * 128
                            if grp == 2:
                                nc.scalar.copy(
                                    xt[:, 8:10, uc:uc + 128],
                                    tps[:, 0:256]
                                    .rearrange("p (a b) -> p a b", a=2))
                                nc.vector.tensor_copy(
                                    xt[0:64, 10, uc:uc + 128],
                                    tps[0:64, 256:384])
                            else:
                                nc.scalar.copy(
                                    xt[:, grp * 4: grp * 4 + 4, uc:uc + 128],
                                    tps[:, 0:512]
                                    .rearrange("p (a b) -> p a b", a=4))
                    xts[dy] = xt
                if dy > 0:
                    xprev = xts.pop(dy - 1)
                    for kb in range(DY_BLOCKS):
                        kw = 64 if kb == 10 else 128
                        for m, ps in ((0, ps0), (1, ps1)):
                            nc.tensor.matmul(
                                ps[:], m0w_sb[0:kw, (dy - 1) * DY_BLOCKS + kb,
                                              m * 128:(m + 1) * 128],
                                xprev[0:kw, kb, :],
                                start=(dy == 1 and kb == 0),
                                stop=(dy == RR and kb == DY_BLOCKS - 1))

            # ---------------- MLP layers 1-3 + head -------------------------
            h0 = hp.tile([128, 2, HID], dt.bfloat16, tag="h")
            nc.scalar.activation(h0[:, 0, :], ps0[:], AF.Relu,
                                 bias=bm_sb[:, 0:1])
            nc.scalar.activation(h0[:, 1, :], ps1[:], AF.Relu,
                                 bias=bm_sb[:, 1:2])
            cur = h0
            for l in (1, 2, 3):
                nxt = hp.tile([128, 2, HID], dt.bfloat16, tag="h")
                for m in range(2):
                    ps = mp2_.tile([128, HID], dt.float32, tag="mlp13ps")
                    for kc in range(2):
                        nc.tensor.matmul(
                            ps[:], m13_sb[:, (l - 1) * 2 + kc,
                                          m * 128:(m + 1) * 128],
                            cur[:, kc, :], start=(kc == 0), stop=(kc == 1))
                    nc.scalar.activation(nxt[:, m, :], ps[:], AF.Relu,
                                         bias=bm_sb[:, 2 * l + m: 2 * l + m + 1])
                cur = nxt
            for u in range(2):
                pso = osp.tile([128, 3], dt.float32, tag="pso")
                for kc in range(2):
                    nc.tensor.matmul(pso[:],
                                     cur[:, kc, u * 128: u * 128 + 128],
                                     m4_sb[:, kc, :],
                                     start=(kc == 0), stop=(kc == 1))
                o1 = ap_.tile([128, 3], dt.float32, tag="o1")
                nc.scalar.copy(o1[:], pso[:])
                nc.gpsimd.tensor_add(o1[:], o1[:], b4_sb[:])
                nc.gpsimd.tensor_add(out_sb[:, t - 1 + u, :], o1[:],
                                     base_all[:, t - 1 + u, :])

        nc.sync.dma_start(
            out.rearrange("(t p) c -> p t c", p=128), out_sb[:])


# ============================ host preparation ==============================

# e-major channel permutation: device channel c' = e*8 + h <- source h*24 + e
PERM = np.array([h * HD + e for e in range(HD) for h in range(HEAD)])


def _host_prep(inputs):
    inp = np.asarray(inputs["inp"], f32)
    sc = np.asarray(inputs["sample_coord"], f32)
    cell = np.asarray(inputs["cell"], f32)

    enc_w = np.asarray(inputs["enc_w"], f32)
    ch_w = np.asarray(inputs["ch_w"], f32)

    w_enc = np.zeros((28, ENC), bf16)
    w_enc[0:27] = enc_w.transpose(1, 2, 3, 0).reshape(27, ENC).astype(bf16)
    w_enc[27] = np.asarray(inputs["enc_b"], f32).astype(bf16)
    w_chp = np.zeros((128, 3, DIM), bf16)
    w_ch2 = np.zeros((ENC, 3, DIM), bf16)
    for ky in range(3):
        w_chp[0:64, ky, :] = ch_w[:, :, ky, 0].T.astype(bf16)
        w_chp[64:128, ky, :] = ch_w[:, :, ky, 1].T.astype(bf16)
        w_ch2[:, ky, :] = ch_w[:, :, ky, 2].T.astype(bf16)

    w_qkv0 = np.zeros((128, 3, 9, DIM), bf16)
    w_qkv1p = np.zeros((128, 3, 3, DIM), bf16)
    w_qkv1k2 = np.zeros((64, 3, 3, DIM), bf16)
    qkvb = np.zeros((128, 6), f32)
    for ci, nm in enumerate(("q", "k", "v")):
        wt = np.asarray(inputs[f"{nm}_w"], f32)
        bt = np.asarray(inputs[f"{nm}_b"], f32)[PERM]
        for off in range(9):
            ky, kx = off // 3, off % 3
            wo = wt[:, :, ky, kx].T[:, PERM]       # [in, out'] e-major
            w_qkv0[:, ci, off, :] = wo[0:128].astype(bf16)
        for ky in range(3):
            wp = wt[PERM][:, 128:192, ky, :]       # [out' e-major, in64, kx]
            w_qkv1p[0:64, ci, ky, :] = wp[:, :, 0].T.astype(bf16)
            w_qkv1p[64:128, ci, ky, :] = wp[:, :, 1].T.astype(bf16)
            w_qkv1k2[:, ci, ky, :] = wp[:, :, 2].T.astype(bf16)
        qkvb[:, ci * 2 + 0] = bt[0:128]
        qkvb[0:64, ci * 2 + 1] = bt[128:192]

    # m0w rows permuted: device xt row = dy*1408 + dx*192 + e*8 + h
    m0w_full = np.asarray(inputs["m0w"], f32)
    DYW_P = DY_BLOCKS * 128
    perm_rows = np.zeros((KBLK * 128, HID), f32)
    for dy in range(RR):
        for dx in range(RR):
            r = dy * RR + dx
            src = m0w_full[r * DIM:(r + 1) * DIM]       # rows h*24+e
            perm_rows[dy * DYW_P + dx * DIM:
                      dy * DYW_P + (dx + 1) * DIM] = src[PERM]
    m0w_dev = np.ascontiguousarray(
        perm_rows.reshape(KBLK, 128, HID).transpose(1, 0, 2)).astype(bf16)

    m13w = np.zeros((128, 6, HID), bf16)
    for l in (1, 2, 3):
        wl = np.asarray(inputs[f"m{l}w"], f32)
        m13w[:, (l - 1) * 2 + 0, :] = wl[0:128].astype(bf16)
        m13w[:, (l - 1) * 2 + 1, :] = wl[128:256].astype(bf16)
    m4w_full = np.asarray(inputs["m4w"], f32)
    m4w = np.stack([m4w_full[0:128], m4w_full[128:256]], 1).astype(bf16)

    b4 = np.broadcast_to(np.asarray(inputs["m4b"], f32)[None, :], (128, 3)).copy()
    ch_bd = np.zeros((128, 2), f32)
    ch_bd[:, 0] = np.asarray(inputs["ch_b"], f32)[0:128]
    ch_bd[0:64, 1] = np.asarray(inputs["ch_b"], f32)[128:192]
    ident = np.eye(128, dtype=bf16)

    m0b = np.asarray(inputs["m0b"], f32)
    m0w_tail = m0w_full[RA * DIM: RA * DIM + 2]
    bias_rest = np.zeros((128, 8), f32)
    for l in (1, 2, 3):
        bl = np.asarray(inputs[f"m{l}b"], f32)
        bias_rest[:, 2 * l + 0] = bl[0:128]
        bias_rest[:, 2 * l + 1] = bl[128:256]

    batch_data = []
    for bi in range(B):
        x = inp[bi]
        xp = np.zeros((CH_IN, H + 8, W + 2), f32)
        xp[:, 4:4 + H, 1:-1] = x          # padded rows: image row r at 4+r

        # fp32 image as two bf16 halves, pixel-block-major [128, 32, 6]
        xT = x.reshape(3, P).T
        hi = xT.astype(bf16).astype(f32)
        lo = (xT - hi).astype(bf16)
        hilo = np.concatenate([hi.astype(bf16), lo], 1)
        hilo = np.ascontiguousarray(
            hilo.reshape(P // 128, 128, 6).transpose(1, 0, 2))

        rel_cell = cell[bi] * np.array([H, W], f32)
        b0 = m0b + rel_cell @ m0w_tail
        bm = bias_rest.copy()
        bm[:, 0] = b0[0:128]
        bm[:, 1] = b0[128:256]
        batch_data.append((x, xp, hilo, bm))

    sqh = f32(1.0 / np.sqrt(HD))
    d = np.arange(-R, R + 1)
    percore = []
    for core in range(N_CORES):
        bi, j = core // 4, core % 4

        # ---- conv strip inputs: im2col for enc rows 16j-2 .. 16j+18 ----
        x, xp, hilo, bm = batch_data[bi]
        col = np.zeros((28, COLN), bf16)
        base_r = SROWS * j - 2
        for c in range(CH_IN):
            for ky in range(3):
                for kx in range(3):
                    # pixel at image row base_r+vr, col cx; neighbor row +ky-1
                    blk = xp[c, base_r + 4 - 1 + ky: base_r + 4 - 1 + ky + EROWS,
                             kx: kx + W]
                    col[c * 9 + ky * 3 + kx] = blk.reshape(-1).astype(bf16)
        rows = base_r + np.arange(EROWS)
        valid_r = ((rows >= 0) & (rows < H))
        col[27] = np.repeat(valid_r, W).astype(bf16)
        rowmask = np.broadcast_to(valid_r.astype(bf16)[None, :],
                                  (128, EROWS)).copy()
        hilo_c = np.ascontiguousarray(hilo[:, 8 * j: 8 * j + 8, :])

        # ---- query-side tables (identical to sample mapping) ----
        qs = slice((core % 4) * QC, ((core % 4) + 1) * QC)
        cy, cx = sc[bi, qs, 0], sc[bi, qs, 1]
        py = (cy + f32(1.0)) * f32(H * 0.5) - f32(0.5)
        px = (cx + f32(1.0)) * f32(W * 0.5) - f32(0.5)
        iy = np.clip(np.floor(py + f32(0.5)), 0, H - 1).astype(np.int64)
        ix = np.clip(np.floor(px + f32(0.5)), 0, W - 1).astype(np.int64)

        dy, dx = [a.reshape(-1) for a in np.meshgrid(d, d, indexing="ij")]
        yy = iy[:, None] + dy[None, :]
        xx = ix[:, None] + dx[None, :]
        valid = ((yy >= 0) & (yy < H) & (xx >= 0) & (xx < W)).astype(f32)
        kvstart = ((iy[:, None] + d[None, :]) * W + ix[:, None] - 3 + KV_PAD)

        y0 = np.floor(py)
        x0 = np.floor(px)
        wy, wx = py - y0, px - x0
        y0 = y0.astype(np.int64)
        x0 = x0.astype(np.int64)
        sy0 = np.clip(y0, 0, H - 2)
        sx0 = np.clip(x0, 0, W - 2)
        wq_eff = np.zeros((QC, 2, 2), f32)
        wb_eff = np.zeros((QC, 2, 2), f32)
        qq = np.arange(QC)
        for ddy, syw in ((0, 1 - wy), (1, wy)):
            for ddx, sxw in ((0, 1 - wx), (1, wx)):
                w = (syw * sxw).astype(f32)
                yc, xc = y0 + ddy, x0 + ddx
                ly = np.clip(yc, 0, H - 1) - sy0
                lx = np.clip(xc, 0, W - 1) - sx0
                wb_eff[qq, ly, lx] += w
                vm = ((yc >= 0) & (yc < H) & (xc >= 0) & (xc < W))
                wq_eff[qq, ly, lx] += w * vm
        cw_ = wb_eff.reshape(QC, 4)
        qstart = ((sy0[:, None] + np.arange(2)[None, :]) * W + sx0[:, None] + Q_PAD)

        kvidx = np.zeros((128, NT, RR * 8), np.int16)
        fq_all = np.concatenate(
            [qstart[t * 128:(t + 1) * 128].T.reshape(-1) for t in range(NT)])
        qidx = np.tile(fq_all.reshape(-1, 16).T, (8, 1)).astype(np.int16)
        maskt = np.zeros((128, NT, RA), f32)
        nmaskt = np.zeros((128, NT), f32)
        qwt = np.zeros((128, NT, 4), f32)
        qwbt = np.zeros((128, NT, 4), f32)
        for t in range(NT):
            ts = slice(t * 128, (t + 1) * 128)
            maskt[:, t, :] = (valid[ts] - 1.0) * 30000.0
            nmaskt[:, t] = RA - valid[ts].sum(axis=1)
            qwt[:, t, :] = wq_eff.reshape(QC, 4)[ts] * sqh
            qwbt[:, t, :] = cw_[ts]
            flat = kvstart[ts].T.reshape(-1)          # j = dy*128 + q
            kvidx[:, t, :] = np.tile(flat.reshape(-1, 16).T, (8, 1)).astype(np.int16)

        percore.append({
            "inp_col": col, "inp_hilo": hilo_c, "rowmask": rowmask,
            "w_enc": w_enc, "w_chp": w_chp, "w_ch2": w_ch2,
            "w_qkv0": w_qkv0, "w_qkv1p": w_qkv1p, "w_qkv1k2": w_qkv1k2,
            "qkvb": qkvb, "ch_b": ch_bd,
            "m0w": m0w_dev, "m13w": m13w, "m4w": m4w, "bmlp": bm, "b4": b4,
            "ident": ident, "kvidx": kvidx, "qidx": qidx, "maskt": maskt,
            "nmaskt": nmaskt,
            "qwt": qwt, "qwbt": qwbt,
        })
    return percore


# ============================== entry point =================================

def _get_program():
    global _PROGRAM
    if _PROGRAM is None:
        _PROGRAM = build_program()
    return _PROGRAM


def kernel(**inputs):
    from concourse import bass_utils
    nc = _get_program()
    in_maps = _host_prep(inputs)
    res = bass_utils.run_bass_kernel_spmd(nc, in_maps, core_ids=list(range(N_CORES)))
    full = np.empty((B, Q, 3), f32)
    for core in range(N_CORES):
        bi, qc = core // 4, core % 4
        full[bi, qc * QC:(qc + 1) * QC] = res.results[core]["out"]
    return full


if __name__ == "__main__":
    import time
    t0 = time.time()
    nc = _get_program()
    print("built+compiled in", time.time() - t0, "s")
